# revision 1
# baseline (speedup 1.0000x reference)
"""Dinov2 SDPA self-attention on one TRN2 chip (8 NeuronCores).

Problem: hidden_states [4, 1370, 1024], 16 heads x 64 dim, fp32.

Sharding (hybrid data/tensor parallel): core c handles batch b = c//2 and
head-group g = c%2 (8 heads = 512 hidden columns). Each core computes its
Q/K/V projections from its batch's activations and runs attention for its
8 heads; the host concatenates the per-core [1370, 512] context outputs.
No on-chip collectives needed.

Per-core algorithm (all layouts transposed so softmax reductions become
matmul contractions):
  Xt = X^T in SBUF  [1024, 1370]
  Qt = Wq_g @ Xt + bq (per-partition bias)   [512, 1370]
  Kt = Wk_g @ Xt  (K bias is softmax-invariant -> dropped exactly)
  V  = X @ Wv_g^T + bv (natural layout, bias via DVE add; softmax weights
       sum to 1 so including bv here is exact)
  per head h: ST = Kt_h^T-tiles @ Qt_h = scores^T  [S, L] (contraction d=64;
       the head pair is emitted as PE row groups 0-63/64-127, though with
       M=128 outputs the PSUM write port serializes them anyway)
  P^T = exp(ST/8) (ACT, fused 1/sqrt(d) scale; no max-subtraction -- scores
       are bounded ~|4|, exp is safe)
  ctxT_ext = [V_h | 1]^T-style stationary @ P^T = [ctx^T; rowsums]  [65, L]
  PE-transpose 128-col slices -> [L_tile, 65], normalize by column 64 via
  DVE reciprocal + per-partition tensor_scalar multiply -> out staging.

Matmul operands are bf16 (fp32 PSUM accumulation); the unnormalized ctx
eviction stays fp32 (bf16 there would round ctx and rowsums independently
-- dominant error term). Validated vs fp32 reference: ~3.5e-3 of absmax.
"""

import os

import numpy as np
import ml_dtypes

import concourse.mybir as mybir
import concourse.tile as tile
from concourse import bacc
from concourse import bass_utils
from concourse.masks import make_identity

F32 = mybir.dt.float32
DT = mybir.dt.bfloat16
NPDT = ml_dtypes.bfloat16
AF = mybir.ActivationFunctionType

B = 4
L = 1370
HID = 1024
NH = 8            # heads per core
D = 64
QD = NH * D       # 512 projected dims per core
HP = NH // 2      # head pairs (PE row-group packing)
KC = HID // 128   # contraction chunks for projections

L_CHUNKS = [(0, 512), (512, 512), (1024, 346)]                      # moving/free dim
TILES = [(i * 128, min(128, L - i * 128)) for i in range((L + 127) // 128)]
NS = len(TILES)   # 11 (last tile 90)


def _body(nc, tc, xt_d, wq_d, wk_d, wv_d, bq_d, bv_d, out_d):
    with tc.tile_pool(name="persist", bufs=1) as pp:
        xt = pp.tile([128, KC, L], DT)
        wq = pp.tile([128, KC, QD], DT)
        wk = pp.tile([128, KC, QD], DT)
        wv = pp.tile([128, KC, QD], DT)
        qt = pp.tile([128, HP, L], DT)
        kt = pp.tile([128, HP, L], DT)
        vv = pp.tile([128, NS, NH, D + 1], DT)   # V tiles + ones column
        ost = pp.tile([128, NS, QD], F32)        # output staging, natural layout
        bqc = pp.tile([128, HP], F32)
        bvb = pp.tile([128, QD], F32)
        ident = pp.tile([128, 128], F32)

        make_identity(nc, ident[:, :])
        nc.vector.memset(vv[:, :, :, D:D + 1], 1.0)

        # Spread input DMAs across the three DGE queues. xt/wv arrive per
        # hidden-chunk so the V-projection's accumulation matmuls start while
        # later chunks are still in flight (also keeps the PE HAM clock warm
        # through the load window instead of idling into a cold start).
        qs = [nc.sync, nc.scalar, nc.gpsimd]
        for k in range(KC):
            r = slice(k * 128, (k + 1) * 128)
            qs[k % 3].dma_start(xt[:, k, :], xt_d[r, :])
            qs[(k + 1) % 3].dma_start(wv[:, k, :], wv_d[r, :])
        nc.scalar.dma_start(wq[:, :, :], wq_d.rearrange("(k p) n -> p k n", p=128))
        nc.sync.dma_start(wk[:, :, :], wk_d.rearrange("(k p) n -> p k n", p=128))
        nc.gpsimd.dma_start(bqc[:, :], bq_d.rearrange("(h p) o -> p (h o)", p=128))
        nc.gpsimd.dma_start(bvb[:, :], bv_d[:, :])

        # ---- fused projection + attention ----
        # One concurrent PSUM layout (8 banks: pq 1 + stAB 2x2 + cAB 2 + tr 1)
        # so Q/K projections for later head pairs and the PE-transposes hide
        # inside the exp-bound attention window of earlier head pairs.
        with (
            tc.tile_pool(name="pqp", bufs=1, space="PSUM") as pqp,
            tc.tile_pool(name="sps", bufs=2, space="PSUM") as sps,
            tc.tile_pool(name="cps", bufs=1, space="PSUM") as cps,
            tc.tile_pool(name="tps", bufs=1, space="PSUM") as tps,
            tc.tile_pool(name="wp", bufs=3) as wp,
        ):
            # Head pipelining: the attention banks are idle until hp0's
            # attention starts, so early projection groups rotate through
            # them instead of serializing on the single proj slot.
            EARLY = ((pqp, "pq"), (sps, "stAB"), (cps, "cAB"), (tps, "tr"))

            for si, (s0, ss) in enumerate(TILES):
                pl, tg = EARLY[si % 4]
                vps = pl.tile([128, QD], F32, name="vps", tag=tg)
                for k in range(KC):
                    nc.tensor.matmul(vps[:ss, :], xt[:, k, s0:s0 + ss], wv[:, k, :],
                                     start=(k == 0), stop=(k == KC - 1))
                nc.vector.tensor_add(
                    vv[:ss, si, :, 0:D],
                    vps[:ss, :].rearrange("p (h d) -> p h d", h=NH),
                    bvb[:ss, :].rearrange("p (h d) -> p h d", h=NH),
                )

            for hp in range(HP):
                m = slice(hp * 128, (hp + 1) * 128)
                for ci, (l0, ln) in enumerate(L_CHUNKS):
                    if hp == 0:
                        pl, tg = EARLY[(2 * ci) % 4]
                        pl2, tg2 = EARLY[(2 * ci + 1) % 4]
                    else:
                        pl, tg = pl2, tg2 = pqp, "pq"
                    qps = pl.tile([128, 512], F32, name="qps", tag=tg)
                    for k in range(KC):
                        nc.tensor.matmul(qps[:, :ln], wq[:, k, m], xt[:, k, l0:l0 + ln],
                                         start=(k == 0), stop=(k == KC - 1))
                    nc.vector.tensor_scalar_add(qt[:, hp, l0:l0 + ln], qps[:, :ln],
                                                bqc[:, hp:hp + 1])
                    kps = pl2.tile([128, 512], F32, name="kps", tag=tg2)
                    for k in range(KC):
                        nc.tensor.matmul(kps[:, :ln], wk[:, k, m], xt[:, k, l0:l0 + ln],
                                         start=(k == 0), stop=(k == KC - 1))
                    nc.vector.tensor_copy(kt[:, hp, l0:l0 + ln], kps[:, :ln])

                hA, hB = 2 * hp, 2 * hp + 1
                for (l0, ln) in L_CHUNKS:
                    cAB = cps.tile([65, 2, 512], F32, name="cAB", tag="cAB")
                    for si, (s0, ss) in enumerate(TILES):
                        stAB = sps.tile([128, 2, 512], F32, name="stAB", tag="stAB")
                        nc.tensor.matmul(stAB[:ss, 0, :ln], kt[0:64, hp, s0:s0 + ss],
                                         qt[0:64, hp, l0:l0 + ln],
                                         start=True, stop=True, tile_position=(0, 0))
                        nc.tensor.matmul(stAB[:ss, 1, :ln], kt[64:128, hp, s0:s0 + ss],
                                         qt[64:128, hp, l0:l0 + ln],
                                         start=True, stop=True, tile_position=(64, 0))
                        eAB = wp.tile([128, 2, 512], DT, name="eAB", tag="eAB")
                        nc.scalar.activation(eAB[:ss, :, :ln], stAB[:ss, :, :ln],
                                             AF.Exp, scale=0.125)
                        nc.tensor.matmul(cAB[:, 0, :ln], vv[:ss, si, hA, :],
                                         eAB[:ss, 0, :ln],
                                         start=(si == 0), stop=(si == NS - 1))
                        nc.tensor.matmul(cAB[:, 1, :ln], vv[:ss, si, hB, :],
                                         eAB[:ss, 1, :ln],
                                         start=(si == 0), stop=(si == NS - 1))
                    ctAB = wp.tile([65, 2, 512], F32, name="ctAB", tag="ctAB")
                    nc.vector.tensor_copy(ctAB[:, :, :ln], cAB[:, :, :ln])
                    for j in range(0, ln, 128):
                        lt = (l0 + j) // 128
                        w = min(128, ln - j)
                        for h2, h in ((0, hA), (1, hB)):
                            # Tail pipelining: banks whose owners are done go
                            # into the transpose rotation. Last unit: proj,
                            # accumulator, and score banks are all free; rest
                            # of the last head pair: proj bank only.
                            if hp == HP - 1 and l0 == L_CHUNKS[-1][0]:
                                pl, tg = ((tps, "tr"), (pqp, "pq"),
                                          (cps, "cAB"), (sps, "stAB"))[
                                    ((j // 128) * 2 + h2) % 4]
                                tr = pl.tile([128, 65], F32, name="trx", tag=tg)
                            elif hp == HP - 1 and (j // 128 + h2) % 2:
                                tr = pqp.tile([128, 65], F32, name="tr2",
                                              tag="pq")
                            else:
                                tr = tps.tile([128, 65], F32, name="tr",
                                              tag="tr")
                            nc.tensor.transpose(tr[:w, :], ctAB[:, h2, j:j + w],
                                                ident[0:65, 0:65])
                            rc = wp.tile([128, 1], F32, name="rc", tag="rc")
                            nc.vector.reciprocal(rc[:w, :], tr[:w, 64:65])
                            nc.vector.tensor_scalar_mul(
                                ost[:w, lt, h * D:(h + 1) * D],
                                tr[:w, 0:D], rc[:w, :])

            for ti, (t0, tn) in enumerate(TILES):
                nc.sync.dma_start(out_d[t0:t0 + tn, :], ost[:tn, ti, :])


_NC_CACHE = {}


def _build(reps=1):
    key = ("nc", reps)
    if key in _NC_CACHE:
        return _NC_CACHE[key]
    nc = bacc.Bacc("TRN2", target_bir_lowering=False, debug=False)
    xt_d = nc.dram_tensor("xt", [HID, L], DT, kind="ExternalInput")
    wq_d = nc.dram_tensor("wqt", [HID, QD], DT, kind="ExternalInput")
    wk_d = nc.dram_tensor("wkt", [HID, QD], DT, kind="ExternalInput")
    wv_d = nc.dram_tensor("wvt", [HID, QD], DT, kind="ExternalInput")
    bq_d = nc.dram_tensor("bq", [QD, 1], F32, kind="ExternalInput")
    bv_d = nc.dram_tensor("bvb", [128, QD], F32, kind="ExternalInput")
    out_d = nc.dram_tensor("out", [L, QD], F32, kind="ExternalOutput")

    with tile.TileContext(nc) as tc:
        for _ in range(reps):
            _body(nc, tc, xt_d.ap(), wq_d.ap(), wk_d.ap(), wv_d.ap(),
                  bq_d.ap(), bv_d.ap(), out_d.ap())
    nc.compile()
    _NC_CACHE[key] = nc
    return nc


def make_in_maps(hidden_states, Wq, bq, Wk, bk, Wv, bv):
    in_maps = []
    for c in range(8):
        b, g = divmod(c, 2)
        gs = slice(g * QD, (g + 1) * QD)
        in_maps.append({
            "xt": np.ascontiguousarray(hidden_states[b].T).astype(NPDT),
            "wqt": np.ascontiguousarray(Wq[gs, :].T).astype(NPDT),
            "wkt": np.ascontiguousarray(Wk[gs, :].T).astype(NPDT),
            "wvt": np.ascontiguousarray(Wv[gs, :].T).astype(NPDT),
            "bq": bq[gs].reshape(QD, 1).astype(np.float32),
            "bvb": np.ascontiguousarray(
                np.broadcast_to(bv[gs], (128, QD))).astype(np.float32),
        })
    return in_maps


LAST_RESULTS = None


def kernel(hidden_states, Wq, bq, Wk, bk, Wv, bv):
    global LAST_RESULTS
    nc = _build()
    in_maps = make_in_maps(hidden_states, Wq, bq, Wk, bk, Wv, bv)
    try:
        res = bass_utils.run_bass_kernel_spmd(
            nc, in_maps, core_ids=list(range(8)),
            trace=bool(os.environ.get("KERNEL_TRACE")),
        )
    except (ImportError, ModuleNotFoundError):
        # The axon NTFF profiling hook is absent in some containers; retry
        # with tracing disabled rather than failing the run.
        prev = os.environ.get("BASS_NEVER_TRACE")
        os.environ["BASS_NEVER_TRACE"] = "1"
        try:
            res = bass_utils.run_bass_kernel_spmd(
                nc, in_maps, core_ids=list(range(8)))
        finally:
            if prev is None:
                os.environ.pop("BASS_NEVER_TRACE", None)
            else:
                os.environ["BASS_NEVER_TRACE"] = prev
    LAST_RESULTS = res
    out = np.empty((B, L, HID), np.float32)
    for c, om in enumerate(res.results):
        b, g = divmod(c, 2)
        out[b, :, g * QD:(g + 1) * QD] = om["out"]
    return out



# revision 41
# speedup vs baseline: 1.3367x; 1.3367x over previous
"""Dinov2 SDPA self-attention on one TRN2 chip (8 NeuronCores).

Problem: hidden_states [4, 1370, 1024], 16 heads x 64 dim, fp32.

Sharding (hybrid data/tensor parallel): core c handles batch b = c//2 and
head-group g = c%2 (8 heads = 512 hidden columns). Each core computes its
Q/K/V projections from its batch's activations and runs attention for its
8 heads; the host concatenates the per-core [1370, 512] context outputs.
No on-chip collectives needed.

v2 layout: context is accumulated in NATURAL [l, d] layout (PSUM out
[l_tile, 65] with exp-scores as the stationary operand), which removes all
PE transposes and the ctx^T staging copies of v1. The V tile carries a
ones column so each ctx PSUM tile ends with the softmax row-sums;
normalization is a batched DVE reciprocal plus per-partition
tensor_scalar_mul into bf16 output staging (host converts to fp32).

Per-core algorithm (all matmul operands bf16, fp32 PSUM):
  Xt = X^T in SBUF  [1024, 1370]
  Qt = Wq_g @ Xt + bq (per-partition bias)   [512, 1370]
  Kt = Wk_g @ Xt  (K bias is softmax-invariant -> dropped exactly)
  V  = X @ Wv_g^T + bv (natural layout; softmax weights sum to 1 so
       including bv here is exact), stored with a ones column -> [s, 65]
  per head pair, per L-chunk, per s-tile:
    ST = Kt_h^T-tiles @ Qt_h = scores^T  [ss, ln]  (PE row groups 0-63/64-127)
    P^T = exp(ST/8) (ACT, fused scale; scores bounded ~|4|, exp safe)
    ctx[l_tile, 0:65] += P^T-slice^T-as-stationary @ [V_h | 1]  (accumulate
       over s-tiles; col 64 = row-sums)
  normalize: rc = 1/ctx[:, 64]; ost[l, h*64:+64] = ctx[:, 0:64] * rc

Q/K/V projection matmuls for later head pairs are emitted as filler inside
the attention instruction stream so the PE never idles while ACT works
through the exp stream (PE ~1.0us and ACT ~1.0us per s-tile group).

Optionally, a subset of s-tiles' exp can run on DVE/Pool via the
Schraudolph bit-trick (exp(x) ~= bitcast_bf16(int16(A*x + B)), one
tensor_scalar op); disabled by default since ACT keeps up.
"""

import os
from collections import deque

import numpy as np
import ml_dtypes

import concourse.mybir as mybir
import concourse.tile as tile
from concourse import bacc
from concourse import bass_utils

F32 = mybir.dt.float32
DT = mybir.dt.bfloat16
I16 = mybir.dt.int16
NPDT = ml_dtypes.bfloat16
AF = mybir.ActivationFunctionType
ALU = mybir.AluOpType

B = 4
L = 1370
HID = 1024
NH = 8            # heads per core
D = 64
QD = NH * D       # 512 projected dims per core
HP = NH // 2      # head pairs (2 heads share the 128 partition rows)
KC = HID // 128   # contraction chunks for projections

L_CHUNKS = [(0, 512), (512, 512), (1024, 346)]
TILES = [(i * 128, min(128, L - i * 128)) for i in range((L + 127) // 128)]
NS = len(TILES)   # 11 (last tile 90)

# Per-head-pair sets of s-tiles whose exp runs on DVE / Pool via the
# Schraudolph bit-trick instead of ACT. Pair 3 carries no projection
# filler, so its PE stream is ACT-paced and gets a bigger offload.
# (GPSIMD/Pool cannot access PSUM, so only DVE can offload score-exp.)
DVE_EXP_STILES = {0: {9}, 1: {9}, 2: {9}, 3: {5, 9}}
POOL_EXP_STILES = {0: set(), 1: set(), 2: set(), 3: set()}
OUT_DMA_ENGINE = "sync"
_LN2 = float(np.log(2.0))
SCH_A = (128.0 / _LN2) * 0.125          # fused 1/sqrt(d) scale
SCH_B = 127.0 * 128.0 - 0.0430357 * 128.0


def _alloc_tiles(nc, tc, stack):
    """Allocate all pools/tiles ONCE for the whole program. Bodies (reps)
    reuse the same tiles so cross-body dependencies stay per-region and the
    next body's input DMAs prefetch during the previous body's tail (a
    fresh pool per body would serialize on the pool boundary)."""
    pp = stack.enter_context(tc.tile_pool(name="persist", bufs=1))
    # Input tiles are double-buffered by body parity so body n+1's loads
    # have no WAR against body n's readers and prefetch a whole body early.
    t = {
        "xt": [pp.tile([128, KC, L], DT, name=f"xt{i}") for i in (0, 1)],
        "wq": [pp.tile([128, KC, QD], DT, name=f"wq{i}") for i in (0, 1)],
        "wk": [pp.tile([128, KC, QD], DT, name=f"wk{i}") for i in (0, 1)],
        "wv": [pp.tile([128, KC, QD], DT, name=f"wv{i}") for i in (0, 1)],
        "bqc": [pp.tile([128, HP], F32, name=f"bqc{i}") for i in (0, 1)],
        "bvb": [pp.tile([128, QD], F32, name=f"bvb{i}") for i in (0, 1)],
        "qt": pp.tile([128, HP, L], DT, name="qt"),
        "kt": pp.tile([128, HP, L], DT, name="kt"),
        "vv": pp.tile([128, NS, NH, D + 1], DT, name="vv"),
        "ost": pp.tile([128, NS, QD], DT, name="ost"),
    }
    t["sps"] = stack.enter_context(tc.tile_pool(name="sps", bufs=2, space="PSUM"))
    t["cpsa"] = stack.enter_context(tc.tile_pool(name="cpsa", bufs=1, space="PSUM"))
    t["cpsb"] = stack.enter_context(tc.tile_pool(name="cpsb", bufs=1, space="PSUM"))
    t["pqp"] = stack.enter_context(tc.tile_pool(name="pqp", bufs=2, space="PSUM"))
    t["wp"] = stack.enter_context(tc.tile_pool(name="wp", bufs=3))
    nc.vector.memset(t["vv"][:, :, :, D:D + 1], 1.0)
    return t


def _load_items(nc, t, par, xt_d, wq_d, wk_d, wv_d, bq_d, bv_d):
    """Input DMA issues as filler items. xt/wv chunk pairs land first (V
    projection consumes them in k order), then wk chunks (pair-0 K before
    Q), then wq and biases. Inputs use only the SP/ACT HWDGE queues;
    outputs go via the gpsimd SWDGE queue so the NEXT body's input loads
    are not stuck behind normalize-gated output stores."""
    xt, wq, wk, wv = t["xt"][par], t["wq"][par], t["wk"][par], t["wv"][par]
    qs = [nc.sync, nc.scalar]
    items = []

    def dma(q, dst, src):
        items.append(lambda: q.dma_start(dst, src))

    for k in range(KC):
        r = slice(k * 128, (k + 1) * 128)
        dma(qs[k % 2], xt[:, k, :], xt_d[r, :])
        dma(qs[k % 2], wv[:, k, :], wv_d[r, :])
    for k in range(KC):
        r = slice(k * 128, (k + 1) * 128)
        dma(qs[k % 2], wk[:, k, :], wk_d[r, :])
    dma(nc.scalar, t["bqc"][par][:, :],
        bq_d.rearrange("(h p) o -> p (h o)", p=128))
    dma(nc.scalar, wq[:, :, :], wq_d.rearrange("(k p) n -> p k n", p=128))
    dma(nc.sync, t["bvb"][par][:, :], bv_d[:, :])
    return items


def _v_proj_items(nc, t, par, si, s0, ss, h0, nh):
    xt, wv, vv, bvb, pqp = (t["xt"][par], t["wv"][par], t["vv"],
                            t["bvb"][par], t["pqp"])
    c0 = h0 * D
    cw = nh * D

    def go():
        vps = pqp.tile([128, QD], F32, name="vps", tag="pq")
        for k in range(KC):
            nc.tensor.matmul(vps[:ss, c0:c0 + cw],
                             xt[:, k, s0:s0 + ss],
                             wv[:, k, c0:c0 + cw],
                             start=(k == 0), stop=(k == KC - 1))
        nc.vector.tensor_add(
            vv[:ss, si, h0:h0 + nh, 0:D],
            vps[:ss, c0:c0 + cw].rearrange("p (h d) -> p h d", h=nh),
            bvb[:ss, c0:c0 + cw].rearrange("p (h d) -> p h d", h=nh),
        )
    return [go]


def _qk_proj_items(nc, t, par, hp, l0, ln, which):
    xt, bqc, pqp = t["xt"][par], t["bqc"][par], t["pqp"]
    m = slice(hp * 128, (hp + 1) * 128)
    w_t, dst = ((t["wq"][par], t["qt"]) if which == "q"
                else (t["wk"][par], t["kt"]))
    box = {}

    def step(k):
        def go():
            if k == 0:
                box["ps"] = pqp.tile([128, 512], F32, name="qkps", tag="pq")
            nc.tensor.matmul(box["ps"][:, :ln], w_t[:, k, m],
                             xt[:, k, l0:l0 + ln],
                             start=(k == 0), stop=(k == KC - 1))
        return go

    def fin():
        if which == "q":
            nc.vector.tensor_scalar_add(dst[:, hp, l0:l0 + ln],
                                        box["ps"][:, :ln], bqc[:, hp:hp + 1])
        else:
            nc.vector.tensor_copy(dst[:, hp, l0:l0 + ln], box["ps"][:, :ln])
    return [step(k) for k in range(KC)] + [fin]


def _prologue_items(nc, t, par, xt_d, wq_d, wk_d, wv_d, bq_d, bv_d):
    """Projection work a body needs before its pair-0 attention: V for
    heads 0-1, the full pair-0 K (scores need all s-tiles), and pair-0 Q
    for chunk 0. Body 0 runs these (after its loads) inline; body n>0's
    items are drained inside body n-1's pair-3 attention (which has no
    projection filler of its own and is otherwise ACT-paced). The input
    DMA items are separate: they only occupy DMA queues, so body n>0's
    loads are emitted a pair earlier (during body n-1's pair-2) and the
    transfers complete while pair-2 finishes."""
    items = []
    for si, (s0, ss) in enumerate(TILES):
        items.extend(_v_proj_items(nc, t, par, si, s0, ss, 0, 2))
    for (l0, ln) in L_CHUNKS:
        items.extend(_qk_proj_items(nc, t, par, 0, l0, ln, "k"))
    items.extend(_qk_proj_items(nc, t, par, 0, 0, 512, "q"))
    return items


def _body(nc, tc, t, par, xt_d, wq_d, wk_d, wv_d, bq_d, bv_d, out_d,
          tail_loads, tail_projs):
    qt, kt, vv, ost = t["qt"], t["kt"], t["vv"], t["ost"]
    sps, cpsa, cpsb, wp = t["sps"], t["cpsa"], t["cpsb"], t["wp"]
    if True:
        if True:
            # The NEXT body's input DMAs go out immediately (double-buffered
            # input tiles -> no WAR): queue-only items, no PE cost.
            for it in tail_loads:
                it()
            # Pair p's attention drains group p: V for the two heads pair
            # p+1 adds, and pair p+1's Q/K projections. Pair 3 drains the
            # next body's prologue projections (V01 + K0 + Q0c0).
            groups = []
            for hp in range(1, HP):
                g = []
                if hp == 1:
                    for (l0, ln) in L_CHUNKS[1:]:
                        g.extend(_qk_proj_items(nc, t, par, 0, l0, ln, "q"))
                for si, (s0, ss) in enumerate(TILES):
                    g.extend(_v_proj_items(nc, t, par, si, s0, ss,
                                           2 * hp, 2))
                for (l0, ln) in L_CHUNKS:
                    g.extend(_qk_proj_items(nc, t, par, hp, l0, ln, "k"))
                    g.extend(_qk_proj_items(nc, t, par, hp, l0, ln, "q"))
                groups.append(deque(g))
            groups.append(deque(tail_projs))
            filler = deque()

            def drain(n):
                for _ in range(n):
                    if filler:
                        filler.popleft()()

            # ---- attention ----
            for hp in range(HP):
                filler = groups[hp]
                for (l0, ln) in L_CHUNKS:
                    jws = [(j, min(128, ln - j * 128))
                           for j in range((ln + 127) // 128)]
                    cA = cpsa.tile([128, 2, 2, D + 1], F32, name="cA", tag="cA")
                    cB = cpsb.tile([128, 2, 2, D + 1], F32, name="cB", tag="cB")

                    # PSUM zero-region semantics: start_tensor_calc lazily
                    # zeroes the whole 2KB bank, so each ctx bank gets ONE
                    # start (first matmul of the chunk into it; the other
                    # slices' first writes land on pending-zero bytes and
                    # overwrite) and ONE stop (last matmul into it).
                    lastj = {0: max(j for j, _ in jws if j < 2),
                             1: max((j for j, _ in jws if j >= 2),
                                    default=None)}

                    def ctx_mm(si, ss, eAB):
                        for h2 in (0, 1):
                            h = 2 * hp + h2
                            for j, w in jws:
                                cp, g = (cA, 0) if j < 2 else (cB, 1)
                                first = (si == 0 and h2 == 0
                                         and j in (0, 2))
                                last = (si == NS - 1 and h2 == 1
                                        and j == lastj[g])
                                nc.tensor.matmul(
                                    cp[:w, h2, j % 2, :],
                                    eAB[:ss, h2, j * 128:j * 128 + w],
                                    vv[:ss, si, h, :],
                                    start=first, stop=last,
                                    skip_group_check=True)

                    # Software pipeline: ctx for s-tile si-1 is emitted after
                    # score/exp of si, so its eAB wait is already (nearly)
                    # satisfied when the PE sequencer reaches it and the
                    # 4-deep wait queue never backs up into the SEQ.
                    prev = None
                    for si, (s0, ss) in enumerate(TILES):
                        stAB = sps.tile([128, 2, 512], F32, name="st", tag="st")
                        nc.tensor.matmul(stAB[:ss, 0, :ln],
                                         kt[0:64, hp, s0:s0 + ss],
                                         qt[0:64, hp, l0:l0 + ln],
                                         start=True, stop=True,
                                         tile_position=(0, 0))
                        nc.tensor.matmul(stAB[:ss, 1, :ln],
                                         kt[64:128, hp, s0:s0 + ss],
                                         qt[64:128, hp, l0:l0 + ln],
                                         start=True, stop=True,
                                         tile_position=(64, 0))
                        eAB = wp.tile([128, 2, 512], DT, name="eAB", tag="eAB")
                        if (si in DVE_EXP_STILES[hp]
                                or si in POOL_EXP_STILES[hp]):
                            eng = (nc.vector if si in DVE_EXP_STILES[hp]
                                   else nc.gpsimd)
                            eng.tensor_scalar(
                                eAB[:ss, :, :ln].bitcast(I16),
                                stAB[:ss, :, :ln], SCH_A, SCH_B,
                                ALU.mult, ALU.add)
                        else:
                            nc.scalar.activation(eAB[:ss, :, :ln],
                                                 stAB[:ss, :, :ln],
                                                 AF.Exp, scale=0.125)
                        if prev is not None:
                            ctx_mm(*prev)
                        prev = (si, ss, eAB)
                        drain(2)
                    ctx_mm(*prev)
                    # normalize: batched reciprocal of the row-sum column,
                    # then per-partition scale into natural-layout staging.
                    nja = min(2, len(jws))
                    njb = len(jws) - nja
                    # Partition range limited to what the ctx matmuls wrote
                    # (the final l-tile is only 90 rows).
                    wB = jws[-1][1] if njb else 128
                    rcA = wp.tile([128, 2, 2, 1], F32, name="rcA", tag="rcA")
                    nc.vector.reciprocal(rcA[:, :, 0:nja, :],
                                         cA[:, :, 0:nja, D:D + 1])
                    if njb:
                        rcB = wp.tile([128, 2, 2, 1], F32, name="rcB",
                                      tag="rcB")
                        nc.vector.reciprocal(rcB[:wB, :, 0:njb, :],
                                             cB[:wB, :, 0:njb, D:D + 1])
                    for j, w in jws:
                        cp, rcx = (cA, rcA) if j < 2 else (cB, rcB)
                        lt = l0 // 128 + j
                        for h2 in (0, 1):
                            h = 2 * hp + h2
                            nc.vector.tensor_scalar_mul(
                                ost[:w, lt, h * D:(h + 1) * D],
                                cp[:w, h2, j % 2, 0:D],
                                rcx[:w, h2, j % 2, :])
                        if hp == HP - 1:
                            # all 8 heads' columns for this l-tile are now
                            # staged -> stream the row block out.
                            getattr(nc, OUT_DMA_ENGINE).dma_start(
                                out_d[lt * 128:lt * 128 + w, :],
                                ost[:w, lt, :])
                drain(len(filler))


_NC_CACHE = {}


def _build(reps=1):
    key = ("nc", reps)
    if key in _NC_CACHE:
        return _NC_CACHE[key]
    nc = bacc.Bacc("TRN2", target_bir_lowering=False, debug=False)
    xt_d = nc.dram_tensor("xt", [HID, L], DT, kind="ExternalInput")
    wq_d = nc.dram_tensor("wqt", [HID, QD], DT, kind="ExternalInput")
    wk_d = nc.dram_tensor("wkt", [HID, QD], DT, kind="ExternalInput")
    wv_d = nc.dram_tensor("wvt", [HID, QD], DT, kind="ExternalInput")
    bq_d = nc.dram_tensor("bq", [QD, 1], F32, kind="ExternalInput")
    bv_d = nc.dram_tensor("bvb", [128, QD], F32, kind="ExternalInput")
    out_d = nc.dram_tensor("out", [L, QD], DT, kind="ExternalOutput")

    from contextlib import ExitStack
    with tile.TileContext(nc) as tc:
        with ExitStack() as stack:
            t = _alloc_tiles(nc, tc, stack)
            aps = (xt_d.ap(), wq_d.ap(), wk_d.ap(), wv_d.ap(),
                   bq_d.ap(), bv_d.ap())
            for it in _load_items(nc, t, 0, *aps):
                it()
            for it in _prologue_items(nc, t, 0, *aps):
                it()
            for rep in range(reps):
                last = rep + 1 >= reps
                par, npar = rep & 1, (rep + 1) & 1
                loads = [] if last else _load_items(nc, t, npar, *aps)
                projs = [] if last else _prologue_items(nc, t, npar, *aps)
                _body(nc, tc, t, par, *aps, out_d.ap(), loads, projs)
    nc.compile()
    _NC_CACHE[key] = nc
    return nc


def make_in_maps(hidden_states, Wq, bq, Wk, bk, Wv, bv):
    in_maps = []
    for c in range(8):
        b, g = divmod(c, 2)
        gs = slice(g * QD, (g + 1) * QD)
        in_maps.append({
            "xt": np.ascontiguousarray(hidden_states[b].T).astype(NPDT),
            "wqt": np.ascontiguousarray(Wq[gs, :].T).astype(NPDT),
            "wkt": np.ascontiguousarray(Wk[gs, :].T).astype(NPDT),
            "wvt": np.ascontiguousarray(Wv[gs, :].T).astype(NPDT),
            "bq": bq[gs].reshape(QD, 1).astype(np.float32),
            "bvb": np.ascontiguousarray(
                np.broadcast_to(bv[gs], (128, QD))).astype(np.float32),
        })
    return in_maps


LAST_RESULTS = None


def kernel(hidden_states, Wq, bq, Wk, bk, Wv, bv):
    global LAST_RESULTS
    nc = _build()
    in_maps = make_in_maps(hidden_states, Wq, bq, Wk, bk, Wv, bv)
    try:
        res = bass_utils.run_bass_kernel_spmd(
            nc, in_maps, core_ids=list(range(8)),
            trace=bool(os.environ.get("KERNEL_TRACE")),
        )
    except (ImportError, ModuleNotFoundError):
        # The axon NTFF profiling hook is absent in some containers; retry
        # with tracing disabled rather than failing the run.
        prev = os.environ.get("BASS_NEVER_TRACE")
        os.environ["BASS_NEVER_TRACE"] = "1"
        try:
            res = bass_utils.run_bass_kernel_spmd(
                nc, in_maps, core_ids=list(range(8)))
        finally:
            if prev is None:
                os.environ.pop("BASS_NEVER_TRACE", None)
            else:
                os.environ["BASS_NEVER_TRACE"] = prev
    LAST_RESULTS = res
    out = np.empty((B, L, HID), np.float32)
    for c, om in enumerate(res.results):
        b, g = divmod(c, 2)
        out[b, :, g * QD:(g + 1) * QD] = om["out"].astype(np.float32)
    return out


# revision 44
# speedup vs baseline: 3.5784x; 2.6771x over previous
"""Dinov2 SDPA self-attention on one TRN2 chip (8 NeuronCores).

Problem: hidden_states [4, 1370, 1024], 16 heads x 64 dim, fp32.

Sharding (hybrid data/tensor parallel): core c handles batch b = c//2 and
head-group g = c%2 (8 heads = 512 hidden columns). Each core computes its
Q/K/V projections from its batch's activations and runs attention for its
8 heads; the host concatenates the per-core [1370, 512] context outputs.
No on-chip collectives needed.

v2 layout: context is accumulated in NATURAL [l, d] layout (PSUM out
[l_tile, 65] with exp-scores as the stationary operand), which removes all
PE transposes and the ctx^T staging copies of v1. The V tile carries a
ones column so each ctx PSUM tile ends with the softmax row-sums;
normalization is a batched DVE reciprocal plus per-partition
tensor_scalar_mul into bf16 output staging (host converts to fp32).

Per-core algorithm (all matmul operands bf16, fp32 PSUM):
  Xt = X^T in SBUF  [1024, 1370]
  Qt = Wq_g @ Xt + bq (per-partition bias)   [512, 1370]
  Kt = Wk_g @ Xt  (K bias is softmax-invariant -> dropped exactly)
  V  = X @ Wv_g^T + bv (natural layout; softmax weights sum to 1 so
       including bv here is exact), stored with a ones column -> [s, 65]
  per head pair, per L-chunk, per s-tile:
    ST = Kt_h^T-tiles @ Qt_h = scores^T  [ss, ln]  (PE row groups 0-63/64-127)
    P^T = exp(ST/8) (ACT, fused scale; scores bounded ~|4|, exp safe)
    ctx[l_tile, 0:65] += P^T-slice^T-as-stationary @ [V_h | 1]  (accumulate
       over s-tiles; col 64 = row-sums)
  normalize: rc = 1/ctx[:, 64]; ost[l, h*64:+64] = ctx[:, 0:64] * rc

Q/K/V projection matmuls for later head pairs are emitted as filler inside
the attention instruction stream so the PE never idles while ACT works
through the exp stream (PE ~1.0us and ACT ~1.0us per s-tile group).

Optionally, a subset of s-tiles' exp can run on DVE/Pool via the
Schraudolph bit-trick (exp(x) ~= bitcast_bf16(int16(A*x + B)), one
tensor_scalar op); disabled by default since ACT keeps up.
"""

import os
from collections import deque

import numpy as np
import ml_dtypes

import concourse.mybir as mybir
import concourse.tile as tile
from concourse import bacc
from concourse import bass_utils

F32 = mybir.dt.float32
DT = mybir.dt.bfloat16
I16 = mybir.dt.int16
NPDT = ml_dtypes.bfloat16
AF = mybir.ActivationFunctionType
ALU = mybir.AluOpType

B = 4
L = 1370
HID = 1024
NH = 8            # heads per core
D = 64
QD = NH * D       # 512 projected dims per core
HP = NH // 2      # head pairs (2 heads share the 128 partition rows)
KC = HID // 128   # contraction chunks for projections

L_CHUNKS = [(0, 512), (512, 512), (1024, 346)]
TILES = [(i * 128, min(128, L - i * 128)) for i in range((L + 127) // 128)]
NS = len(TILES)   # 11 (last tile 90)

# Per-head-pair sets of s-tiles whose exp runs on DVE / Pool via the
# Schraudolph bit-trick instead of ACT. Pair 3 carries no projection
# filler, so its PE stream is ACT-paced and gets a bigger offload.
# Schraudolph exp offload to DVE: the cost model likes it, but on real HW
# TensorScalarPtr is slower than modeled and it LOSES ~40us -> disabled.
# (GPSIMD/Pool cannot access PSUM at all, so Pool cannot help either.)
DVE_EXP_STILES = {0: set(), 1: set(), 2: set(), 3: set()}
POOL_EXP_STILES = {0: set(), 1: set(), 2: set(), 3: set()}
OUT_DMA_ENGINE = "sync"
_LN2 = float(np.log(2.0))
SCH_A = (128.0 / _LN2) * 0.125          # fused 1/sqrt(d) scale
SCH_B = 127.0 * 128.0 - 0.0430357 * 128.0


def _alloc_tiles(nc, tc, stack):
    """Allocate all pools/tiles ONCE for the whole program. Bodies (reps)
    reuse the same tiles so cross-body dependencies stay per-region and the
    next body's input DMAs prefetch during the previous body's tail (a
    fresh pool per body would serialize on the pool boundary)."""
    pp = stack.enter_context(tc.tile_pool(name="persist", bufs=1))
    # Input tiles are double-buffered by body parity so body n+1's loads
    # have no WAR against body n's readers and prefetch a whole body early.
    t = {
        "xt": [pp.tile([128, KC, L], DT, name=f"xt{i}") for i in (0, 1)],
        "wq": [pp.tile([128, KC, QD], DT, name=f"wq{i}") for i in (0, 1)],
        "wk": [pp.tile([128, KC, QD], DT, name=f"wk{i}") for i in (0, 1)],
        "wv": [pp.tile([128, KC, QD], DT, name=f"wv{i}") for i in (0, 1)],
        "bqc": [pp.tile([128, HP], F32, name=f"bqc{i}") for i in (0, 1)],
        "bvb": [pp.tile([128, QD], F32, name=f"bvb{i}") for i in (0, 1)],
        "qt": pp.tile([128, HP, L], DT, name="qt"),
        "kt": pp.tile([128, HP, L], DT, name="kt"),
        "vv": pp.tile([128, NS, NH, D + 1], DT, name="vv"),
        "ost": pp.tile([128, NS, QD], DT, name="ost"),
    }
    t["sps"] = stack.enter_context(tc.tile_pool(name="sps", bufs=2, space="PSUM"))
    t["cpsa"] = stack.enter_context(tc.tile_pool(name="cpsa", bufs=1, space="PSUM"))
    t["cpsb"] = stack.enter_context(tc.tile_pool(name="cpsb", bufs=1, space="PSUM"))
    t["pqp"] = stack.enter_context(tc.tile_pool(name="pqp", bufs=2, space="PSUM"))
    t["wp"] = stack.enter_context(tc.tile_pool(name="wp", bufs=3))
    nc.vector.memset(t["vv"][:, :, :, D:D + 1], 1.0)
    return t


def _load_items(nc, t, par, xt_d, wq_d, wk_d, wv_d, bq_d, bv_d):
    """Input DMA issues as filler items. xt/wv chunk pairs land first (V
    projection consumes them in k order), then wk chunks (pair-0 K before
    Q), then wq and biases. Inputs use only the SP/ACT HWDGE queues;
    outputs go via the gpsimd SWDGE queue so the NEXT body's input loads
    are not stuck behind normalize-gated output stores."""
    xt, wq, wk, wv = t["xt"][par], t["wq"][par], t["wk"][par], t["wv"][par]
    qs = [nc.sync, nc.scalar]
    items = []

    def dma(q, dst, src):
        items.append(lambda: q.dma_start(dst, src))

    for k in range(KC):
        r = slice(k * 128, (k + 1) * 128)
        dma(qs[k % 2], xt[:, k, :], xt_d[r, :])
        dma(qs[k % 2], wv[:, k, :], wv_d[r, :])
    for k in range(KC):
        r = slice(k * 128, (k + 1) * 128)
        dma(qs[k % 2], wk[:, k, :], wk_d[r, :])
    dma(nc.scalar, t["bqc"][par][:, :],
        bq_d.rearrange("(h p) o -> p (h o)", p=128))
    dma(nc.scalar, wq[:, :, :], wq_d.rearrange("(k p) n -> p k n", p=128))
    dma(nc.sync, t["bvb"][par][:, :], bv_d[:, :])
    return items


def _v_proj_items(nc, t, par, si, s0, ss, h0, nh):
    xt, wv, vv, bvb, pqp = (t["xt"][par], t["wv"][par], t["vv"],
                            t["bvb"][par], t["pqp"])
    c0 = h0 * D
    cw = nh * D

    def go():
        vps = pqp.tile([128, QD], F32, name="vps", tag="pq")
        for k in range(KC):
            nc.tensor.matmul(vps[:ss, c0:c0 + cw],
                             xt[:, k, s0:s0 + ss],
                             wv[:, k, c0:c0 + cw],
                             start=(k == 0), stop=(k == KC - 1))
        nc.vector.tensor_add(
            vv[:ss, si, h0:h0 + nh, 0:D],
            vps[:ss, c0:c0 + cw].rearrange("p (h d) -> p h d", h=nh),
            bvb[:ss, c0:c0 + cw].rearrange("p (h d) -> p h d", h=nh),
        )
    return [go]


def _qk_proj_items(nc, t, par, hp, l0, ln, which):
    xt, bqc, pqp = t["xt"][par], t["bqc"][par], t["pqp"]
    m = slice(hp * 128, (hp + 1) * 128)
    w_t, dst = ((t["wq"][par], t["qt"]) if which == "q"
                else (t["wk"][par], t["kt"]))
    box = {}

    def step(k):
        def go():
            if k == 0:
                box["ps"] = pqp.tile([128, 512], F32, name="qkps", tag="pq")
            nc.tensor.matmul(box["ps"][:, :ln], w_t[:, k, m],
                             xt[:, k, l0:l0 + ln],
                             start=(k == 0), stop=(k == KC - 1))
        return go

    def fin():
        if which == "q":
            nc.vector.tensor_scalar_add(dst[:, hp, l0:l0 + ln],
                                        box["ps"][:, :ln], bqc[:, hp:hp + 1])
        else:
            nc.vector.tensor_copy(dst[:, hp, l0:l0 + ln], box["ps"][:, :ln])
    return [step(k) for k in range(KC)] + [fin]


def _prologue_items(nc, t, par, xt_d, wq_d, wk_d, wv_d, bq_d, bv_d):
    """Projection work a body needs before its pair-0 attention: V for
    heads 0-1, the full pair-0 K (scores need all s-tiles), and pair-0 Q
    for chunk 0. Body 0 runs these (after its loads) inline; body n>0's
    items are drained inside body n-1's pair-3 attention (which has no
    projection filler of its own and is otherwise ACT-paced). The input
    DMA items are separate: they only occupy DMA queues, so body n>0's
    loads are emitted a pair earlier (during body n-1's pair-2) and the
    transfers complete while pair-2 finishes."""
    items = []
    for si, (s0, ss) in enumerate(TILES):
        items.extend(_v_proj_items(nc, t, par, si, s0, ss, 0, 2))
    for (l0, ln) in L_CHUNKS:
        items.extend(_qk_proj_items(nc, t, par, 0, l0, ln, "k"))
    items.extend(_qk_proj_items(nc, t, par, 0, 0, 512, "q"))
    return items


def _emit_ctx(nc, t, hp, item, ctx_mm, normalize, cpsa, cpsb):
    ch, si, ss, eAB = item
    if si == 0:
        ch["cA"] = cpsa.tile([128, 2, 2, D + 1], F32, name="cA", tag="cA")
        ch["cB"] = cpsb.tile([128, 2, 2, D + 1], F32, name="cB", tag="cB")
    ctx_mm(hp, ch, si, ss, eAB)
    if si == NS - 1:
        normalize(hp, ch)


def _body(nc, tc, t, par, xt_d, wq_d, wk_d, wv_d, bq_d, bv_d, out_d,
          tail_loads, tail_projs):
    qt, kt, vv, ost = t["qt"], t["kt"], t["vv"], t["ost"]
    sps, cpsa, cpsb, wp = t["sps"], t["cpsa"], t["cpsb"], t["wp"]
    if True:
        if True:
            # The NEXT body's input DMAs go out immediately (double-buffered
            # input tiles -> no WAR): queue-only items, no PE cost.
            for it in tail_loads:
                it()
            # Pair p's attention drains group p: V for the two heads pair
            # p+1 adds, and pair p+1's Q/K projections. Pair 3 drains the
            # next body's prologue projections (V01 + K0 + Q0c0).
            groups = []
            for hp in range(1, HP):
                g = []
                if hp == 1:
                    for (l0, ln) in L_CHUNKS[1:]:
                        g.extend(_qk_proj_items(nc, t, par, 0, l0, ln, "q"))
                for si, (s0, ss) in enumerate(TILES):
                    g.extend(_v_proj_items(nc, t, par, si, s0, ss,
                                           2 * hp, 2))
                for (l0, ln) in L_CHUNKS:
                    g.extend(_qk_proj_items(nc, t, par, hp, l0, ln, "k"))
                    g.extend(_qk_proj_items(nc, t, par, hp, l0, ln, "q"))
                groups.append(deque(g))
            groups.append(deque(tail_projs))
            filler = deque()

            def drain(n):
                for _ in range(n):
                    if filler:
                        filler.popleft()()

            # ---- attention ----
            # Each pair runs a single flat stream over (chunk, s-tile):
            # score/exp of stream slot g overlap ctx of slot g-2, so ctx's
            # eAB wait is satisfied when the PE sequencer reaches it and
            # the pipeline crosses chunk seams without stalling.
            def ctx_mm(hp, ch, si, ss, eAB):
                cA, cB, jws = ch["cA"], ch["cB"], ch["jws"]
                for h2 in (0, 1):
                    h = 2 * hp + h2
                    for j, w in jws:
                        cp, g = (cA, 0) if j < 2 else (cB, 1)
                        # PSUM zero-region semantics: start_tensor_calc
                        # lazily zeroes the whole 2KB bank, so each ctx
                        # bank gets ONE start (first matmul of the chunk
                        # into it; the other slices' first writes land on
                        # pending-zero bytes and overwrite) and ONE stop.
                        first = (si == 0 and h2 == 0 and j in (0, 2))
                        last = (si == NS - 1 and h2 == 1
                                and j == ch["lastj"][g])
                        nc.tensor.matmul(
                            cp[:w, h2, j % 2, :],
                            eAB[:ss, h2, j * 128:j * 128 + w],
                            vv[:ss, si, h, :],
                            start=first, stop=last,
                            skip_group_check=True)

            def normalize(hp, ch):
                cA, cB, jws, l0 = ch["cA"], ch["cB"], ch["jws"], ch["l0"]
                nja = min(2, len(jws))
                njb = len(jws) - nja
                # Partition range limited to what the ctx matmuls wrote
                # (the final l-tile is only 90 rows).
                wB = jws[-1][1] if njb else 128
                rcA = wp.tile([128, 2, 2, 1], F32, name="rcA", tag="rcA")
                nc.vector.reciprocal(rcA[:, :, 0:nja, :],
                                     cA[:, :, 0:nja, D:D + 1])
                if njb:
                    rcB = wp.tile([128, 2, 2, 1], F32, name="rcB", tag="rcB")
                    nc.vector.reciprocal(rcB[:wB, :, 0:njb, :],
                                         cB[:wB, :, 0:njb, D:D + 1])
                for j, w in jws:
                    cp, rcx = (cA, rcA) if j < 2 else (cB, rcB)
                    lt = l0 // 128 + j
                    for h2 in (0, 1):
                        h = 2 * hp + h2
                        nc.vector.tensor_scalar_mul(
                            ost[:w, lt, h * D:(h + 1) * D],
                            cp[:w, h2, j % 2, 0:D],
                            rcx[:w, h2, j % 2, :])
                    if hp == HP - 1:
                        # all 8 heads' columns for this l-tile are now
                        # staged -> stream the row block out.
                        getattr(nc, OUT_DMA_ENGINE).dma_start(
                            out_d[lt * 128:lt * 128 + w, :],
                            ost[:w, lt, :])

            for hp in range(HP):
                filler = groups[hp]
                chunks = []
                for (l0, ln) in L_CHUNKS:
                    jws = [(j, min(128, ln - j * 128))
                           for j in range((ln + 127) // 128)]
                    chunks.append({
                        "l0": l0, "ln": ln, "jws": jws,
                        "lastj": {0: max(j for j, _ in jws if j < 2),
                                  1: max((j for j, _ in jws if j >= 2),
                                         default=None)},
                        "cA": None, "cB": None,
                    })
                stream = [(ch, si, s0, ss)
                          for ch in chunks
                          for si, (s0, ss) in enumerate(TILES)]
                pend = deque()
                for ch, si, s0, ss in stream:
                    l0, ln = ch["l0"], ch["ln"]
                    stAB = sps.tile([128, 2, 512], F32, name="st", tag="st")
                    nc.tensor.matmul(stAB[:ss, 0, :ln],
                                     kt[0:64, hp, s0:s0 + ss],
                                     qt[0:64, hp, l0:l0 + ln],
                                     start=True, stop=True,
                                     tile_position=(0, 0))
                    nc.tensor.matmul(stAB[:ss, 1, :ln],
                                     kt[64:128, hp, s0:s0 + ss],
                                     qt[64:128, hp, l0:l0 + ln],
                                     start=True, stop=True,
                                     tile_position=(64, 0))
                    eAB = wp.tile([128, 2, 512], DT, name="eAB", tag="eAB")
                    if si in DVE_EXP_STILES[hp]:
                        nc.vector.tensor_scalar(
                            eAB[:ss, :, :ln].bitcast(I16),
                            stAB[:ss, :, :ln], SCH_A, SCH_B,
                            ALU.mult, ALU.add)
                    else:
                        nc.scalar.activation(eAB[:ss, :, :ln],
                                             stAB[:ss, :, :ln],
                                             AF.Exp, scale=0.125)
                    pend.append((ch, si, ss, eAB))
                    if len(pend) > 2:
                        _emit_ctx(nc, t, hp, pend.popleft(), ctx_mm,
                                  normalize, cpsa, cpsb)
                    drain(2)
                while pend:
                    _emit_ctx(nc, t, hp, pend.popleft(), ctx_mm,
                              normalize, cpsa, cpsb)
                drain(len(filler))


_NC_CACHE = {}


def _build(reps=1):
    key = ("nc", reps)
    if key in _NC_CACHE:
        return _NC_CACHE[key]
    nc = bacc.Bacc("TRN2", target_bir_lowering=False, debug=False)
    xt_d = nc.dram_tensor("xt", [HID, L], DT, kind="ExternalInput")
    wq_d = nc.dram_tensor("wqt", [HID, QD], DT, kind="ExternalInput")
    wk_d = nc.dram_tensor("wkt", [HID, QD], DT, kind="ExternalInput")
    wv_d = nc.dram_tensor("wvt", [HID, QD], DT, kind="ExternalInput")
    bq_d = nc.dram_tensor("bq", [QD, 1], F32, kind="ExternalInput")
    bv_d = nc.dram_tensor("bvb", [128, QD], F32, kind="ExternalInput")
    out_d = nc.dram_tensor("out", [L, QD], DT, kind="ExternalOutput")

    from contextlib import ExitStack
    with tile.TileContext(nc) as tc:
        with ExitStack() as stack:
            t = _alloc_tiles(nc, tc, stack)
            aps = (xt_d.ap(), wq_d.ap(), wk_d.ap(), wv_d.ap(),
                   bq_d.ap(), bv_d.ap())
            for it in _load_items(nc, t, 0, *aps):
                it()
            for it in _prologue_items(nc, t, 0, *aps):
                it()
            for rep in range(reps):
                last = rep + 1 >= reps
                par, npar = rep & 1, (rep + 1) & 1
                loads = [] if last else _load_items(nc, t, npar, *aps)
                projs = [] if last else _prologue_items(nc, t, npar, *aps)
                _body(nc, tc, t, par, *aps, out_d.ap(), loads, projs)
    nc.compile()
    _NC_CACHE[key] = nc
    return nc


def make_in_maps(hidden_states, Wq, bq, Wk, bk, Wv, bv):
    in_maps = []
    for c in range(8):
        b, g = divmod(c, 2)
        gs = slice(g * QD, (g + 1) * QD)
        in_maps.append({
            "xt": np.ascontiguousarray(hidden_states[b].T).astype(NPDT),
            "wqt": np.ascontiguousarray(Wq[gs, :].T).astype(NPDT),
            "wkt": np.ascontiguousarray(Wk[gs, :].T).astype(NPDT),
            "wvt": np.ascontiguousarray(Wv[gs, :].T).astype(NPDT),
            "bq": bq[gs].reshape(QD, 1).astype(np.float32),
            "bvb": np.ascontiguousarray(
                np.broadcast_to(bv[gs], (128, QD))).astype(np.float32),
        })
    return in_maps


LAST_RESULTS = None


def kernel(hidden_states, Wq, bq, Wk, bk, Wv, bv):
    global LAST_RESULTS
    nc = _build()
    in_maps = make_in_maps(hidden_states, Wq, bq, Wk, bk, Wv, bv)
    try:
        res = bass_utils.run_bass_kernel_spmd(
            nc, in_maps, core_ids=list(range(8)),
            trace=bool(os.environ.get("KERNEL_TRACE")),
        )
    except (ImportError, ModuleNotFoundError):
        # The axon NTFF profiling hook is absent in some containers; retry
        # with tracing disabled rather than failing the run.
        prev = os.environ.get("BASS_NEVER_TRACE")
        os.environ["BASS_NEVER_TRACE"] = "1"
        try:
            res = bass_utils.run_bass_kernel_spmd(
                nc, in_maps, core_ids=list(range(8)))
        finally:
            if prev is None:
                os.environ.pop("BASS_NEVER_TRACE", None)
            else:
                os.environ["BASS_NEVER_TRACE"] = prev
    LAST_RESULTS = res
    out = np.empty((B, L, HID), np.float32)
    for c, om in enumerate(res.results):
        b, g = divmod(c, 2)
        out[b, :, g * QD:(g + 1) * QD] = om["out"].astype(np.float32)
    return out
